# revision 3
# baseline (speedup 1.0000x reference)
"""AllToAllDispatchBackward (MoE dispatch) Trainium2 kernel.

Reference computes: out[d, t, :] = input[t, :] if token t routed to device d
(via either of its top-2 experts), else 0.  Shapes: input [8192, 4096] f32,
expert_indices [8192, 2] i32, expert_mapping [64] i32, out [8, 8192, 4096] f32.

Sharding: tokens are data-parallel across the 8 cores (1024 tokens each).
Each core loads each of its token tiles [128, 4096] once and produces the
8 per-device output slices for those tokens by multiplying with a
per-partition routing mask (0/1), writing [8, 1024, 4096].  Host computes
the tiny routing mask from the index tensors (the sharding decision) and
concatenates the per-core results along T.  Per-core HBM traffic:
16 MiB read + 128 MiB write.
"""

import numpy as np

T, H, E, K = 8192, 4096, 64, 2
D = 8  # device slices in the output (ROUTING_ROWS)
NCORES = 8
TS = T // NCORES  # tokens per core = 1024
P = 128  # SBUF partitions
NT = TS // P  # token tiles per core = 8

TRACE = False  # test harness can flip this to profile
LAST_RESULT = None  # BassKernelResults from the most recent run

_CACHE = {}


def _build_nc():
    import concourse.bacc as bacc
    import concourse.mybir as mybir
    from concourse.tile import TileContext

    nc = bacc.Bacc(
        "TRN2",
        target_bir_lowering=False,
        debug=False,
        enable_asserts=False,
        num_devices=NCORES,
    )
    x = nc.dram_tensor("x", [TS, H], mybir.dt.float32, kind="ExternalInput")
    m = nc.dram_tensor("m", [P, D * NT], mybir.dt.float32, kind="ExternalInput")
    y = nc.dram_tensor("y", [D, TS, H], mybir.dt.float32, kind="ExternalOutput")

    with TileContext(nc) as tc:
        with (
            tc.tile_pool(name="mask", bufs=1) as mpool,
            tc.tile_pool(name="xin", bufs=3) as xpool,
            tc.tile_pool(name="out", bufs=8) as opool,
        ):
            mt = mpool.tile([P, D * NT], mybir.dt.float32)
            nc.sync.dma_start(out=mt[:], in_=m[:])
            for j in range(NT):
                xt = xpool.tile([P, H], mybir.dt.float32)
                nc.sync.dma_start(out=xt[:], in_=x[j * P : (j + 1) * P, :])
                for d in range(D):
                    c = d * NT + j
                    ot = opool.tile([P, H], mybir.dt.float32)
                    nc.vector.tensor_scalar_mul(
                        out=ot[:], in0=xt[:], scalar1=mt[:, c : c + 1]
                    )
                    nc.scalar.dma_start(out=y[d, j * P : (j + 1) * P, :], in_=ot[:])
    nc.compile()
    return nc


def kernel(input_tensor, expert_indices, expert_mapping):
    global LAST_RESULT
    from concourse.bass_utils import run_bass_kernel_spmd

    if "nc" not in _CACHE:
        _CACHE["nc"] = _build_nc()
    nc = _CACHE["nc"]

    x = np.ascontiguousarray(np.asarray(input_tensor), dtype=np.float32)
    ei = np.asarray(expert_indices)
    em = np.asarray(expert_mapping)

    # Routing mask [D, T]: token t goes to device d via any of its K experts.
    tok_dev = em[ei]  # [T, K]
    mask = np.zeros((D, T), dtype=np.float32)
    tt = np.arange(T)
    for k in range(tok_dev.shape[1]):
        mask[tok_dev[:, k], tt] = 1.0

    in_maps = []
    for c in range(NCORES):
        sl = slice(c * TS, (c + 1) * TS)
        # mc[p, d*NT + j] = mask[d, c*TS + j*P + p]
        mc = mask[:, sl].reshape(D, NT, P).transpose(2, 0, 1).reshape(P, D * NT)
        in_maps.append({"x": x[sl], "m": np.ascontiguousarray(mc)})

    res = run_bass_kernel_spmd(
        nc, in_maps, core_ids=list(range(NCORES)), trace=TRACE
    )
    LAST_RESULT = res
    return np.concatenate([r["y"] for r in res.results], axis=1)
